# revision 4
# baseline (speedup 1.0000x reference)
"""Link-predictor GNN kernel for 8 TRN2 NeuronCores.

Strategy (per sharding hint): shard edges across 8 cores (data parallel),
replicate the bf16-cast node-embedding table + MLP weights on every core.

Per core (75264 edges = 147 tiles x 512 edges, 21 gather-chunks x 3584):
  1. SWDGE indirect gather: 3584 embedding rows/call (bf16, 256B rows),
     landing [128 lanes, 28 subtiles x 128 d] in SBUF.
  2. PE transpose (bf16, via identity) each [128e,128d] subtile into PSUM
     -> X^T layout [128 d, 512 e]; DVE copies PSUM->SBUF.
  3. matmul1: h[128h, 512e] (2 halves) = W1_blk^T . X^T, K=2x128 accum.
  4. ACT: relu(h + b1) -> bf16 SBUF.
  5. matmul2: logits[1, 512] = W2_blk^T . h, K=2x128 accum.
  6. ACT: sigmoid(logits + b2) -> f32 SBUF; HWDGE DMA to DRAM out.
"""

import os
import sys

sys.path.insert(0, "/opt/trn_rl_repo")

import numpy as np
import ml_dtypes

from concourse import bacc, mybir, tile
from concourse.bass import IndirectOffsetOnAxis
from concourse.bass_utils import run_bass_kernel_spmd

BF16 = ml_dtypes.bfloat16

N_NODES = 100000
D = 128
H = 256
E_TOTAL = 600000
NCORES = 8
E_CORE = 75000          # real edges per core
TILE_E = 512            # edges per compute tile
TILES_PER_CHUNK = 7
SUB = 4 * TILES_PER_CHUNK          # 28 gather subtiles (128 edges) per chunk
CHUNK_E = SUB * 128                # 3584 edges per gather chunk
CHUNKS = 21
EC_PAD = CHUNKS * CHUNK_E          # 75264 padded edges per core
NT = CHUNKS * TILES_PER_CHUNK      # 147 tiles

LAST_RESULTS = None
_NC = None


def _build_program():
    global _NC
    if _NC is not None:
        return _NC
    dt = mybir.dt
    nc = bacc.Bacc(
        "TRN2",
        target_bir_lowering=False,
        debug=False,
        enable_asserts=False,
        num_devices=NCORES,
    )
    emd = nc.dram_tensor("emd", [N_NODES, D], dt.bfloat16, kind="ExternalInput")
    soff_d = nc.dram_tensor("soff", [128, CHUNKS * SUB], dt.int32, kind="ExternalInput")
    doff_d = nc.dram_tensor("doff", [128, CHUNKS * SUB], dt.int32, kind="ExternalInput")
    w1_d = nc.dram_tensor("w1", [128, 512], dt.bfloat16, kind="ExternalInput")
    w2_d = nc.dram_tensor("w2", [128, 2], dt.bfloat16, kind="ExternalInput")
    b1_d = nc.dram_tensor("b1", [128, 2], dt.float32, kind="ExternalInput")
    b2_d = nc.dram_tensor("b2", [1, 1], dt.float32, kind="ExternalInput")
    ident_d = nc.dram_tensor("ident", [128, 128], dt.bfloat16, kind="ExternalInput")
    out_d = nc.dram_tensor("out", [NT, TILE_E], dt.float32, kind="ExternalOutput")

    AF = mybir.ActivationFunctionType

    with tile.TileContext(nc) as tc:
        with (
            tc.tile_pool(name="const", bufs=1) as cpool,
            tc.tile_pool(name="g", bufs=2) as gpool,
            tc.tile_pool(name="x", bufs=3) as xpool,
            tc.tile_pool(name="h", bufs=3) as hpool,
            tc.tile_pool(name="o", bufs=4) as opool,
            tc.tile_pool(name="px", bufs=2, space="PSUM") as pxp,
            tc.tile_pool(name="ph", bufs=2, space="PSUM") as php,
            tc.tile_pool(name="pl", bufs=2, space="PSUM") as plp,
        ):
            w1_sb = cpool.tile([128, 512], dt.bfloat16)
            nc.sync.dma_start(w1_sb[:, :], w1_d[:, :])
            w2_sb = cpool.tile([128, 2], dt.bfloat16)
            nc.sync.dma_start(w2_sb[:, :], w2_d[:, :])
            b1_sb = cpool.tile([128, 2], dt.float32)
            nc.sync.dma_start(b1_sb[:, :], b1_d[:, :])
            b2_sb = cpool.tile([1, 1], dt.float32)
            nc.sync.dma_start(b2_sb[:, :], b2_d[:, :])
            ident = cpool.tile([128, 128], dt.bfloat16)
            nc.sync.dma_start(ident[:, :], ident_d[:, :])
            soff = cpool.tile([128, CHUNKS * SUB], dt.int32)
            nc.sync.dma_start(soff[:, :], soff_d[:, :])
            doff = cpool.tile([128, CHUNKS * SUB], dt.int32)
            nc.sync.dma_start(doff[:, :], doff_d[:, :])

            for c in range(CHUNKS):
                g_s = gpool.tile([128, CHUNK_E], dt.bfloat16, tag="gs")
                g_d = gpool.tile([128, CHUNK_E], dt.bfloat16, tag="gd")
                # HW walrus indirect DMA consumes exactly one index per
                # partition (128 rows/call) — one call per 128-edge subtile.
                for m in range(SUB):
                    col = c * SUB + m
                    nc.gpsimd.indirect_dma_start(
                        out=g_s[:, m * 128 : (m + 1) * 128],
                        out_offset=None,
                        in_=emd[:, :],
                        in_offset=IndirectOffsetOnAxis(
                            ap=soff[:, col : col + 1], axis=0
                        ),
                    )
                    nc.gpsimd.indirect_dma_start(
                        out=g_d[:, m * 128 : (m + 1) * 128],
                        out_offset=None,
                        in_=emd[:, :],
                        in_offset=IndirectOffsetOnAxis(
                            ap=doff[:, col : col + 1], axis=0
                        ),
                    )
                for t in range(TILES_PER_CHUNK):
                    T = c * TILES_PER_CHUNK + t
                    # transpose 4 src + 4 dst subtiles into one PSUM tile:
                    # cols 0:512 = Xsrc^T, cols 512:1024 = Xdst^T
                    x_ps = pxp.tile([128, 1024], dt.bfloat16, tag="xps")
                    for i in range(4):
                        m = t * 4 + i
                        nc.tensor.transpose(
                            out=x_ps[:, i * 128 : (i + 1) * 128],
                            in_=g_s[:, m * 128 : (m + 1) * 128],
                            identity=ident[:, :],
                        )
                        nc.tensor.transpose(
                            out=x_ps[:, 512 + i * 128 : 512 + (i + 1) * 128],
                            in_=g_d[:, m * 128 : (m + 1) * 128],
                            identity=ident[:, :],
                        )
                    x_sb = xpool.tile([128, 1024], dt.bfloat16, tag="xsb")
                    nc.vector.tensor_copy(out=x_sb[:, :], in_=x_ps[:, :])

                    h0_ps = php.tile([128, 512], dt.float32, tag="h0")
                    h1_ps = php.tile([128, 512], dt.float32, tag="h1")
                    # h = Xsrc @ W1[:128] + Xdst @ W1[128:]
                    nc.tensor.matmul(
                        h0_ps[:, :], lhsT=w1_sb[:, 0:128], rhs=x_sb[:, 0:512],
                        start=True, stop=False,
                    )
                    nc.tensor.matmul(
                        h0_ps[:, :], lhsT=w1_sb[:, 256:384], rhs=x_sb[:, 512:1024],
                        start=False, stop=True,
                    )
                    nc.tensor.matmul(
                        h1_ps[:, :], lhsT=w1_sb[:, 128:256], rhs=x_sb[:, 0:512],
                        start=True, stop=False,
                    )
                    nc.tensor.matmul(
                        h1_ps[:, :], lhsT=w1_sb[:, 384:512], rhs=x_sb[:, 512:1024],
                        start=False, stop=True,
                    )
                    h0_sb = hpool.tile([128, 512], dt.bfloat16, tag="h0sb")
                    h1_sb = hpool.tile([128, 512], dt.bfloat16, tag="h1sb")
                    nc.scalar.activation(
                        h0_sb[:, :], h0_ps[:, :], AF.Relu, bias=b1_sb[:, 0:1]
                    )
                    nc.scalar.activation(
                        h1_sb[:, :], h1_ps[:, :], AF.Relu, bias=b1_sb[:, 1:2]
                    )
                    l_ps = plp.tile([1, TILE_E], dt.float32, tag="lps")
                    nc.tensor.matmul(
                        l_ps[:, :], lhsT=w2_sb[:, 0:1], rhs=h0_sb[:, :],
                        start=True, stop=False,
                    )
                    nc.tensor.matmul(
                        l_ps[:, :], lhsT=w2_sb[:, 1:2], rhs=h1_sb[:, :],
                        start=False, stop=True,
                    )
                    o_sb = opool.tile([1, TILE_E], dt.float32, tag="osb")
                    nc.scalar.activation(
                        o_sb[:, :], l_ps[:, :], AF.Sigmoid, bias=b2_sb[:, 0:1]
                    )
                    nc.sync.dma_start(out_d[T : T + 1, :], o_sb[:, :])

    nc.compile()
    _NC = nc
    return nc


def _arrange_offsets(idx):
    """[EC_PAD] int32 -> [128, CHUNKS*SUB] so that offs[q, c*SUB+m] is the
    node index of edge c*CHUNK_E + m*128 + q."""
    return np.ascontiguousarray(
        idx.reshape(CHUNKS, SUB, 128).transpose(2, 0, 1).reshape(128, CHUNKS * SUB)
    )


def _prepare_inputs(emd_all, edge_index, W1, b1, W2, b2):
    emd_bf = np.ascontiguousarray(np.asarray(emd_all, dtype=np.float32)).astype(BF16)
    ei = np.asarray(edge_index).astype(np.int32)
    W1 = np.asarray(W1, dtype=np.float32)
    W2 = np.asarray(W2, dtype=np.float32)
    b1 = np.asarray(b1, dtype=np.float32).reshape(-1)
    b2 = np.asarray(b2, dtype=np.float32).reshape(-1)

    # lhsT blocks: cols 0:256 = W1[:128,:] (src side), 256:512 = W1[128:,:]
    w1_arr = np.concatenate([W1[:D, :], W1[D:, :]], axis=1).astype(BF16)
    w2_arr = np.stack([W2[:128, 0], W2[128:, 0]], axis=1).astype(BF16)
    b1_arr = np.ascontiguousarray(np.stack([b1[:128], b1[128:]], axis=1))
    b2_arr = b2.reshape(1, 1)
    ident = np.eye(128, dtype=np.float32).astype(BF16)

    in_maps = []
    for c in range(NCORES):
        sl = ei[c * E_CORE : (c + 1) * E_CORE]
        src = np.zeros(EC_PAD, np.int32)
        dst = np.zeros(EC_PAD, np.int32)
        src[: E_CORE] = sl[:, 0]
        dst[: E_CORE] = sl[:, 1]
        in_maps.append(
            {
                "emd": emd_bf,
                "soff": _arrange_offsets(src),
                "doff": _arrange_offsets(dst),
                "w1": w1_arr,
                "w2": w2_arr,
                "b1": b1_arr,
                "b2": b2_arr,
                "ident": ident,
            }
        )
    return in_maps


def kernel(emd_all, edge_index, W1, b1, W2, b2):
    global LAST_RESULTS
    in_maps = _prepare_inputs(emd_all, edge_index, W1, b1, W2, b2)
    nc = _build_program()
    res = run_bass_kernel_spmd(nc, in_maps, core_ids=list(range(NCORES)))
    LAST_RESULTS = res
    outs = [
        np.asarray(res.results[c]["out"], dtype=np.float32).reshape(-1)[:E_CORE]
        for c in range(NCORES)
    ]
    return np.concatenate(outs).reshape(E_TOTAL, 1)


if __name__ == "__main__":
    rng = np.random.default_rng(0)
    emd = rng.standard_normal((N_NODES, D), dtype=np.float32)
    ei = rng.integers(0, N_NODES, size=(E_TOTAL, 2)).astype(np.int32)
    W1 = rng.standard_normal((2 * D, H), dtype=np.float32) / np.sqrt(2 * D)
    W2 = rng.standard_normal((H, 1), dtype=np.float32) / np.sqrt(H)
    out = kernel(emd, ei, W1, np.zeros(H, np.float32), W2, np.zeros(1, np.float32))
    print(out.shape, out[:4, 0])
